# revision 13
# baseline (speedup 1.0000x reference)
"""Trainium2 Bass kernel for gated 1x1-conv attention (dense_transformer).

Problem structure (B=4, C=3, H=W=64, heads=3 => c_h=1):
  - attention logits are rank-1: att[n,m] = softmax_m(q_n * k_m), N=4096
  - luma gate multiplies q; 1x1 convs are 3x3 channel mixes
Sharding: 8 cores = (batch b = j//2) x (query-pixel half = j%2).
Each core computes the full output for its 2048 query pixels (all 3 heads),
so no collectives are needed.

v1 (direct): for each head, S[m_tile, n] = exp(k_m * q'_n) via ScalarE
activation with per-partition scale, reduced over m by TensorE matmuls with
stationary [v_m, 1] columns, accumulating [num_n, den_n] in PSUM.
"""

import numpy as np

import concourse.bass as bass
import concourse.bacc as bacc
import concourse.mybir as mybir
from concourse.tile import TileContext
from concourse.bass_utils import run_bass_kernel_spmd

F32 = mybir.dt.float32
BF16 = mybir.dt.bfloat16
AF = mybir.ActivationFunctionType
ALU = mybir.AluOpType

N = 4096          # pixels per image
NSL = 2048        # query pixels per core
NMT = 32          # m tiles of 128
P = 128
NCHUNK = 4        # n chunks of 512 for matmul free dim
LUMW = (0.299, 0.587, 0.114)


def build_nc(debug=False):
    nc = bacc.Bacc("TRN2", target_bir_lowering=False, debug=False,
                   num_devices=8)

    img = nc.declare_dram_parameter("img", [3, N], F32, isOutput=False)
    qimg = nc.declare_dram_parameter("qimg", [3, NSL], F32, isOutput=False)
    wkvl = nc.declare_dram_parameter("wkvl", [3, 8], F32, isOutput=False)
    wqT = nc.declare_dram_parameter("wqT", [3, 3], F32, isOutput=False)
    lumrep = nc.declare_dram_parameter("lumrep", [3, 3], F32, isOutput=False)
    woT = nc.declare_dram_parameter("woT", [3, 3], F32, isOutput=False)
    out = nc.declare_dram_parameter("out", [3, NSL], F32, isOutput=True)
    if debug:
        dbg_q = nc.declare_dram_parameter("dbg_q", [3, NSL], F32, isOutput=True)
        dbg_g = nc.declare_dram_parameter("dbg_g", [3, NSL], F32, isOutput=True)
        dbg_k = nc.declare_dram_parameter("dbg_k", [P, NMT * 3], F32, isOutput=True)
        dbg_nd = nc.declare_dram_parameter("dbg_nd", [2, NSL], F32, isOutput=True)
        dbg_st = nc.declare_dram_parameter("dbg_st", [3, 4], F32, isOutput=True)
        dbg_att = nc.declare_dram_parameter("dbg_att", [3, NSL], F32, isOutput=True)
        dbg_fo = nc.declare_dram_parameter("dbg_fo", [3, NSL], F32, isOutput=True)

    with TileContext(nc) as tc:
        with (
            tc.tile_pool(name="singles", bufs=1) as singles,
            tc.tile_pool(name="sbuf", bufs=2) as sb,
            tc.tile_pool(name="rows", bufs=2) as rows,
            tc.tile_pool(name="hrow", bufs=4) as hrow,
            tc.tile_pool(name="stile", bufs=3) as stile,
            tc.tile_pool(name="qb", bufs=2) as qbpool,
            tc.tile_pool(name="psum", bufs=1, space="PSUM") as ps,
            tc.tile_pool(name="psum_acc", bufs=1, space="PSUM") as psacc,
            tc.tile_pool(name="dram", bufs=1, space="DRAM") as dpool,
        ):
            # ---- load inputs ----
            img_sb = singles.tile([3, N], F32)
            nc.sync.dma_start(out=img_sb[:], in_=img[:])
            qimg_sb = singles.tile([3, NSL], F32)
            nc.sync.dma_start(out=qimg_sb[:], in_=qimg[:])
            wkvl_sb = singles.tile([3, 8], F32)
            nc.sync.dma_start(out=wkvl_sb[:], in_=wkvl[:])
            wqT_sb = singles.tile([3, 3], F32)
            nc.sync.dma_start(out=wqT_sb[:], in_=wqT[:])
            lumrep_sb = singles.tile([3, 3], F32)
            nc.sync.dma_start(out=lumrep_sb[:], in_=lumrep[:])
            woT_sb = singles.tile([3, 3], F32)
            nc.sync.dma_start(out=woT_sb[:], in_=woT[:])

            ones_1x128 = singles.tile([1, P], F32)
            nc.vector.memset(ones_1x128[:], 1.0)
            ones_128x3 = singles.tile([P, 3], F32)
            nc.vector.memset(ones_128x3[:], 1.0)

            # ---- conv pass: k, v, L for all m tiles ----
            # psum_conv[:, mt, 0:3]=k heads, 3:6=v heads, 6=L
            psum_conv = ps.tile([P, NMT, 8], F32, tag="big")
            for mt in range(NMT):
                nc.tensor.matmul(
                    psum_conv[:, mt, :],
                    lhsT=img_sb[:, mt * P:(mt + 1) * P],
                    rhs=wkvl_sb[:],
                    start=True, stop=True,
                )
            k_sb = singles.tile([P, NMT, 3], F32)
            nc.vector.tensor_copy(k_sb[:], psum_conv[:, :, 0:3])
            # lw[mt, h, :] = [v_h, 1] in bf16 (stationary operand for reduce)
            lw_sb = singles.tile([P, NMT, 3, 2], BF16)
            nc.vector.tensor_copy(lw_sb[:, :, :, 0], psum_conv[:, :, 3:6])
            nc.vector.memset(lw_sb[:, :, :, 1], 1.0)
            L_sb = singles.tile([P, NMT], F32)
            nc.vector.tensor_copy(L_sb[:], psum_conv[:, :, 6])

            # ---- luma stats over the full image ----
            Lr = sb.tile([P, 2], F32)
            nc.vector.tensor_reduce(Lr[:, 0:1], L_sb[:], axis=mybir.AxisListType.X,
                                    op=ALU.add)
            mu1_ps = ps.tile([1, 1], F32, tag="big")
            nc.tensor.matmul(mu1_ps[:], lhsT=ones_128x3[:, 0:1], rhs=Lr[:, 0:1],
                             start=True, stop=True)
            mu_sb = sb.tile([1, 1], F32)
            nc.vector.tensor_scalar_mul(mu_sb[:], mu1_ps[:], 1.0 / N)
            mu128_ps = ps.tile([P, 1], F32, tag="big")
            nc.tensor.matmul(mu128_ps[:], lhsT=ones_1x128[:], rhs=mu_sb[:],
                             start=True, stop=True)

            mu128_sb = sb.tile([P, 1], F32)
            nc.vector.tensor_copy(mu128_sb[:], mu128_ps[:])
            dltmp = sb.tile([P, NMT], F32)
            nc.vector.tensor_scalar(dltmp[:], L_sb[:], mu128_sb[:, 0:1], None,
                                    op0=ALU.subtract)
            sr = sb.tile([P, 2], F32)
            nc.vector.tensor_reduce(sr[:, 0:1], dltmp[:], axis=mybir.AxisListType.X,
                                    op=ALU.add, apply_absolute_value=True)
            dl2 = sb.tile([P, NMT], F32)
            nc.vector.tensor_tensor(dl2[:], dltmp[:], dltmp[:], op=ALU.mult)
            nc.vector.tensor_reduce(sr[:, 1:2], dl2[:], axis=mybir.AxisListType.X,
                                    op=ALU.add)
            stats_ps = ps.tile([3, 2], F32, tag="big")
            nc.tensor.matmul(stats_ps[:], lhsT=ones_128x3[:], rhs=sr[:],
                             start=True, stop=True)

            stats_sb = sb.tile([3, 2], F32)
            nc.vector.tensor_copy(stats_sb[:], stats_ps[:])
            s1sq = sb.tile([3, 1], F32)
            nc.vector.tensor_tensor(s1sq[:], stats_sb[:, 0:1], stats_sb[:, 0:1],
                                    op=ALU.mult)
            var_sb = sb.tile([3, 1], F32)
            # var = (sum2 - sum1^2/N) / (N-1); dL mean is sum1/N
            nc.vector.scalar_tensor_tensor(var_sb[:], in0=s1sq[:], scalar=-1.0 / N,
                                           in1=stats_sb[:, 1:2],
                                           op0=ALU.mult, op1=ALU.add)
            nc.vector.tensor_scalar_mul(var_sb[:], var_sb[:], 1.0 / (N - 1))
            # std = sqrt(var) + 1e-6 via exp(0.5*ln(var)) (same ACT table set)
            lnv = sb.tile([3, 1], F32)
            nc.scalar.activation(lnv[:], var_sb[:], AF.Ln)
            stdv = sb.tile([3, 1], F32)
            nc.scalar.activation(stdv[:], lnv[:], AF.Exp, scale=0.5)
            nc.vector.tensor_scalar_add(stdv[:], stdv[:], 1e-6)
            rneg = sb.tile([3, 1], F32)
            nc.vector.reciprocal(rneg[:], stdv[:])
            nc.vector.tensor_scalar_mul(rneg[:], rneg[:], -1.0)

            # ---- gate + q' rows at the query slice ----
            Lq_ps = ps.tile([3, NSL], F32, tag="big")
            for ch in range(NCHUNK):
                nc.tensor.matmul(Lq_ps[:, ch * 512:(ch + 1) * 512],
                                 lhsT=lumrep_sb[:],
                                 rhs=qimg_sb[:, ch * 512:(ch + 1) * 512],
                                 start=True, stop=True)
            dlq = rows.tile([3, NSL], F32, tag="grow")
            nc.vector.tensor_scalar(dlq[:], Lq_ps[:], mu128_sb[0:3, 0:1], None,
                                    op0=ALU.subtract)
            # |x| = max(-x, x)
            nc.vector.scalar_tensor_tensor(dlq[:], in0=dlq[:], scalar=-1.0,
                                           in1=dlq[:], op0=ALU.mult, op1=ALU.max)
            eg = rows.tile([3, NSL], F32, tag="grow")
            nc.scalar.activation(eg[:], dlq[:], AF.Exp, scale=rneg[:])
            nc.vector.tensor_scalar_add(eg[:], eg[:], 1.0)
            gr = rows.tile([3, NSL], F32, tag="grow")
            nc.vector.reciprocal(gr[:], eg[:])
            nc.vector.tensor_scalar_add(gr[:], gr[:], 1.0)  # 1 + gate

            q_ps = ps.tile([3, NSL], F32, tag="big")
            for ch in range(NCHUNK):
                nc.tensor.matmul(q_ps[:, ch * 512:(ch + 1) * 512],
                                 lhsT=wqT_sb[:],
                                 rhs=qimg_sb[:, ch * 512:(ch + 1) * 512],
                                 start=True, stop=True)
            qrow_sb = rows.tile([3, NSL], F32, tag="grow")
            nc.vector.tensor_tensor(qrow_sb[:], q_ps[:], gr[:], op=ALU.mult)

            if debug:
                nc.sync.dma_start(out=dbg_q[:], in_=qrow_sb[:])
                nc.sync.dma_start(out=dbg_g[:], in_=gr[:])
                nc.sync.dma_start(out=dbg_k[:], in_=k_sb[:])
                dbgst_sb = sb.tile([3, 4], F32)
                nc.vector.tensor_copy(dbgst_sb[:, 0:1], var_sb[:])
                nc.vector.tensor_copy(dbgst_sb[:, 1:2], rneg[:])
                nc.vector.tensor_copy(dbgst_sb[:, 2:3], mu128_sb[0:3, :])
                nc.vector.tensor_copy(dbgst_sb[:, 3:4], stats_sb[:, 0:1])
                nc.sync.dma_start(out=dbg_st[:], in_=dbgst_sb[:])
            # stage q' rows to DRAM for partition-broadcast reload
            qscratch = dpool.tile([3, NSL], F32)
            nc.sync.dma_start(out=qscratch[:], in_=qrow_sb[:])

            # ---- per-head attention ----
            att_sb = singles.tile([3, NSL], F32)
            for h in range(3):
                qb_sb = qbpool.tile([P, NSL], F32)
                nc.sync.dma_start(out=qb_sb[:],
                                  in_=qscratch[h:h + 1, :].partition_broadcast(P))
                psum_h = psacc.tile([2, NSL], F32)
                for mt in range(NMT):
                    s_t = stile.tile([P, NSL], BF16)
                    nc.scalar.activation(s_t[:], qb_sb[:], AF.Exp,
                                         scale=k_sb[:, mt, h:h + 1])
                    for ch in range(NCHUNK):
                        nc.tensor.matmul(
                            psum_h[:, ch * 512:(ch + 1) * 512],
                            lhsT=lw_sb[:, mt, h, :],
                            rhs=s_t[:, ch * 512:(ch + 1) * 512],
                            start=(mt == 0), stop=(mt == NMT - 1),
                        )
                nd_sb = hrow.tile([2, NSL], F32, tag="hrow")
                nc.vector.tensor_copy(nd_sb[:], psum_h[:])
                den0 = hrow.tile([1, NSL], F32, tag="hrow")
                nc.sync.dma_start(out=den0[:], in_=nd_sb[1:2, :])
                rden = hrow.tile([1, NSL], F32, tag="hrow")
                nc.vector.reciprocal(rden[:], den0[:])
                att_h = hrow.tile([1, NSL], F32, tag="hrow")
                nc.vector.tensor_tensor(att_h[:], nd_sb[0:1, :],
                                        rden[:], op=ALU.mult)
                nc.sync.dma_start(out=att_sb[h:h + 1, :], in_=att_h[:])
                if debug and h == 0:
                    nc.sync.dma_start(out=dbg_nd[:], in_=nd_sb[:])

            # ---- output mix + residual + clip ----
            fo_ps = ps.tile([3, NSL], F32, tag="big")
            for ch in range(NCHUNK):
                nc.tensor.matmul(fo_ps[:, ch * 512:(ch + 1) * 512],
                                 lhsT=woT_sb[:],
                                 rhs=att_sb[:, ch * 512:(ch + 1) * 512],
                                 start=True, stop=True)
            res_sb = rows.tile([3, NSL], F32, tag="grow")
            if debug:
                nc.sync.dma_start(out=dbg_att[:], in_=att_sb[:])
                fo_dbg = rows.tile([3, NSL], F32, tag="grow")
                nc.vector.tensor_copy(fo_dbg[:], fo_ps[:])
                nc.sync.dma_start(out=dbg_fo[:], in_=fo_dbg[:])
            nc.vector.tensor_tensor(res_sb[:], fo_ps[:], qimg_sb[:], op=ALU.add)
            res2_sb = rows.tile([3, NSL], F32, tag="grow")
            nc.vector.tensor_scalar_max(res2_sb[:], res_sb[:], 0.0)
            res3_sb = rows.tile([3, NSL], F32, tag="grow")
            nc.vector.tensor_scalar_min(res3_sb[:], res2_sb[:], 1.0)
            nc.sync.dma_start(out=out[:], in_=res3_sb[:])

    nc.finalize()
    return nc


_NC_CACHE = {}


def _get_nc():
    if "nc" not in _NC_CACHE:
        _NC_CACHE["nc"] = build_nc()
    return _NC_CACHE["nc"]


def make_in_maps(rgb, wq, wk, wv, wo):
    x = np.ascontiguousarray(rgb.reshape(4, 3, N)).astype(np.float32)
    lumw = np.array(LUMW, dtype=np.float32)
    wkvl = np.concatenate(
        [wk.T, wv.T, lumw[:, None], np.zeros((3, 1), np.float32)], axis=1
    ).astype(np.float32)
    wqT = np.ascontiguousarray(wq.T).astype(np.float32)
    lumrep = np.tile(lumw[:, None], (1, 3)).astype(np.float32)
    woT = np.ascontiguousarray(wo.T).astype(np.float32)

    in_maps = []
    for j in range(8):
        b, half = j // 2, j % 2
        sl = slice(half * NSL, (half + 1) * NSL)
        in_maps.append({
            "img": x[b],
            "qimg": np.ascontiguousarray(x[b][:, sl]),
            "wkvl": wkvl,
            "wqT": wqT,
            "lumrep": lumrep,
            "woT": woT,
        })
    return in_maps


def run(rgb, wq, wk, wv, wo, trace=False):
    nc = _get_nc()
    in_maps = make_in_maps(rgb, wq, wk, wv, wo)
    res = run_bass_kernel_spmd(nc, in_maps, core_ids=list(range(8)), trace=trace)
    y = np.zeros((4, 3, N), dtype=np.float32)
    for j in range(8):
        b, half = j // 2, j % 2
        sl = slice(half * NSL, (half + 1) * NSL)
        y[b][:, sl] = res.results[j]["out"]
    return y.reshape(4, 3, 64, 64), res


def kernel(**inputs):
    y, _ = run(inputs["rgb"], inputs["wq"], inputs["wk"], inputs["wv"],
               inputs["wo"])
    return y


# revision 19
# speedup vs baseline: 2.4247x; 2.4247x over previous
"""Trainium2 Bass kernel for gated 1x1-conv attention (dense_transformer).

Problem structure (B=4, C=3, H=W=64, heads=3 => c_h=1): attention logits are
rank-1: att[n] = softmax_m(q_n * k_m) @ v, N=4096 pixels. A luma gate scales
q; the 1x1 convs are 3x3 channel mixes.

Sharding: 8 cores = (batch b = j//2) x (query-pixel half = j%2); each core
produces the full RGB output for its 2048 query pixels. No collectives.

v2 algorithm (Gaussian-quadrature factorization of the exp kernel):
  exp(q k) = e^{-s^2 k^2/2} * (h/(s sqrt(2pi))) * sum_j e^{-(q-t_j)^2/(2s^2)} e^{t_j k}
over a T=128 grid t_j. This collapses the N x N attention into N x T + T x N
work (constants cancel in the softmax ratio):
  grid:  gnum[j] = sum_m (c_m v_m) e^{t_j k_m},  gden[j] = sum_m c_m e^{t_j k_m}
         with c_m = e^{-s^2 k_m^2 / 2}   (ScalarE exp + TensorE matmul)
  rbf:   W[j, n] = e^{-(q_n - t_j)^2/(2 s^2)}    (ScalarE square+exp)
         att[n] = (W.T @ gnum) / (W.T @ gden)    (TensorE)
Max rel err vs exact softmax ~4e-3 (bf16 matmuls), verified in numpy.
"""

import numpy as np

import concourse.bass as bass
import concourse.bacc as bacc
import concourse.mybir as mybir
from concourse.tile import TileContext
from concourse.bass_utils import run_bass_kernel_spmd

F32 = mybir.dt.float32
BF16 = mybir.dt.bfloat16
AF = mybir.ActivationFunctionType
ALU = mybir.AluOpType

N = 4096          # pixels per image
NSL = 2048        # query pixels per core
NMT = 32          # key (m) tiles of 128
NQT = 16          # query tiles of 128
P = 128
T = 128           # Gaussian-quadrature grid size
T0, T1 = -2.6, 1.7
HG = (T1 - T0) / (T - 1)
SIG = 1.25 * HG
ISQ = 1.0 / (SIG * np.sqrt(2.0))   # 1/(sigma*sqrt(2))
LUMW = (0.299, 0.587, 0.114)


def build_nc(debug=False):
    nc = bacc.Bacc("TRN2", target_bir_lowering=False, debug=False,
                   num_devices=8)

    img = nc.declare_dram_parameter("img", [3, N], F32, isOutput=False)
    qimg = nc.declare_dram_parameter("qimg", [3, NSL], F32, isOutput=False)
    qimgT = nc.declare_dram_parameter("qimgT", [P, 3 * NQT], F32, isOutput=False)
    wkvl = nc.declare_dram_parameter("wkvl", [3, 8], F32, isOutput=False)
    wql = nc.declare_dram_parameter("wql", [3, 4], F32, isOutput=False)
    wocol = nc.declare_dram_parameter("wocol", [P, 9], F32, isOutput=False)
    tbc = nc.declare_dram_parameter("tbc", [P, T], F32, isOutput=False)
    tsig = nc.declare_dram_parameter("tsig", [P, 1], F32, isOutput=False)
    sigk = nc.declare_dram_parameter("sigk", [P, 1], F32, isOutput=False)
    isq = nc.declare_dram_parameter("isq", [P, 1], F32, isOutput=False)
    out = nc.declare_dram_parameter("out", [P, 3 * NQT], F32, isOutput=True)
    if debug:
        dbg_g = nc.declare_dram_parameter("dbg_g", [P, 6], F32, isOutput=True)
        dbg_att = nc.declare_dram_parameter("dbg_att", [P, 3 * NQT], F32,
                                            isOutput=True)
        dbg_qp = nc.declare_dram_parameter("dbg_qp", [P, 3 * NQT], F32,
                                           isOutput=True)
        dbg_nd = nc.declare_dram_parameter("dbg_nd", [P, 3 * 2 * NQT], F32,
                                           isOutput=True)
        dbg_w = nc.declare_dram_parameter("dbg_w", [P, NSL], F32,
                                          isOutput=True)

    with TileContext(nc) as tc:
        with (
            tc.tile_pool(name="singles", bufs=1) as singles,
            tc.tile_pool(name="sbuf", bufs=2) as sb,
            tc.tile_pool(name="stile", bufs=3) as stile,
            tc.tile_pool(name="wtile", bufs=2) as wtile,
            tc.tile_pool(name="psum", bufs=1, space="PSUM") as ps,
            tc.tile_pool(name="psum_g", bufs=2, space="PSUM") as psg,
            tc.tile_pool(name="psum_att", bufs=2, space="PSUM") as psa,
            tc.tile_pool(name="psum_qb", bufs=1, space="PSUM") as psq,
            tc.tile_pool(name="dram", bufs=1, space="DRAM") as dpool,
        ):
            # ---- load inputs ----
            img_sb = singles.tile([3, N], F32)
            nc.sync.dma_start(out=img_sb[:], in_=img[:])
            qimg_sb = singles.tile([3, NSL], F32)
            nc.sync.dma_start(out=qimg_sb[:], in_=qimg[:])
            qimgT_sb = singles.tile([P, 3 * NQT], F32)
            nc.sync.dma_start(out=qimgT_sb[:], in_=qimgT[:])
            wkvl_sb = singles.tile([3, 8], F32)
            nc.sync.dma_start(out=wkvl_sb[:], in_=wkvl[:])
            wql_sb = singles.tile([3, 4], F32)
            nc.sync.dma_start(out=wql_sb[:], in_=wql[:])
            wocol_sb = singles.tile([P, 9], F32)
            nc.sync.dma_start(out=wocol_sb[:], in_=wocol[:])
            tbc_sb = singles.tile([P, T], F32)
            nc.sync.dma_start(out=tbc_sb[:], in_=tbc[:])
            tsig_sb = singles.tile([P, 1], F32)
            nc.sync.dma_start(out=tsig_sb[:], in_=tsig[:])
            sigk_sb = singles.tile([P, 1], F32)
            nc.sync.dma_start(out=sigk_sb[:], in_=sigk[:])
            isq_sb = singles.tile([P, 1], F32)
            nc.sync.dma_start(out=isq_sb[:], in_=isq[:])

            ones_1x128 = singles.tile([1, P], F32)
            nc.vector.memset(ones_1x128[:], 1.0)
            ones_sq = singles.tile([P, P], F32)
            nc.vector.memset(ones_sq[:], 1.0)

            # ---- conv pass over keys: k, v, L columns ----
            psum_conv = ps.tile([P, NMT, 8], F32, tag="big")
            for mt in range(NMT):
                nc.tensor.matmul(
                    psum_conv[:, mt, :],
                    lhsT=img_sb[:, mt * P:(mt + 1) * P],
                    rhs=wkvl_sb[:],
                    start=True, stop=True,
                )
            k_sb = singles.tile([P, NMT, 3], F32)
            nc.vector.tensor_copy(k_sb[:], psum_conv[:, :, 0:3])
            L_sb = singles.tile([P, NMT], F32)
            nc.vector.tensor_copy(L_sb[:], psum_conv[:, :, 6])

            # c = exp(-(k*sig)^2/2); stationary pairs lw2[mt,h,:] = [c*v, c]
            csq = sb.tile([P, NMT, 3], F32)
            nc.scalar.activation(csq[:], k_sb[:], AF.Square,
                                 scale=sigk_sb[:, 0:1])
            c_sb = sb.tile([P, NMT, 3], F32)
            nc.scalar.activation(c_sb[:], csq[:], AF.Exp, scale=-1.0)
            lw2_sb = singles.tile([P, NMT, 3, 2], BF16)
            nc.vector.tensor_tensor(lw2_sb[:, :, :, 0], psum_conv[:, :, 3:6],
                                    c_sb[:], op=ALU.mult)
            nc.vector.tensor_copy(lw2_sb[:, :, :, 1], c_sb[:])

            # ---- conv pass over queries: q cols + Lq col ----
            psum_q = ps.tile([P, NQT, 4], F32, tag="big2")
            for qt in range(NQT):
                nc.tensor.matmul(
                    psum_q[:, qt, :],
                    lhsT=qimg_sb[:, qt * P:(qt + 1) * P],
                    rhs=wql_sb[:],
                    start=True, stop=True,
                )

            # ---- luma stats (replicated to all 128 partitions) ----
            Lr = sb.tile([P, 1], F32)
            nc.vector.tensor_reduce(Lr[:], L_sb[:], axis=mybir.AxisListType.X,
                                    op=ALU.add)
            mu_ps = psg.tile([P, 1], F32, tag="g")
            nc.tensor.matmul(mu_ps[:], lhsT=ones_sq[:], rhs=Lr[:],
                             start=True, stop=True)
            mu_sb = sb.tile([P, 1], F32)
            nc.vector.tensor_scalar_mul(mu_sb[:], mu_ps[:], 1.0 / N)

            dltmp = sb.tile([P, NMT], F32)
            nc.vector.tensor_scalar(dltmp[:], L_sb[:], mu_sb[:, 0:1], None,
                                    op0=ALU.subtract)
            sr = sb.tile([P, 2], F32)
            nc.vector.tensor_reduce(sr[:, 0:1], dltmp[:],
                                    axis=mybir.AxisListType.X,
                                    op=ALU.add, apply_absolute_value=True)
            dl2 = sb.tile([P, NMT], F32)
            nc.vector.tensor_tensor(dl2[:], dltmp[:], dltmp[:], op=ALU.mult)
            nc.vector.tensor_reduce(sr[:, 1:2], dl2[:],
                                    axis=mybir.AxisListType.X, op=ALU.add)
            stats_ps = psg.tile([P, 2], F32, tag="g")
            nc.tensor.matmul(stats_ps[:], lhsT=ones_sq[:], rhs=sr[:],
                             start=True, stop=True)
            stats_sb = sb.tile([P, 2], F32)
            nc.vector.tensor_copy(stats_sb[:], stats_ps[:])
            s1sq = sb.tile([P, 1], F32)
            nc.vector.tensor_tensor(s1sq[:], stats_sb[:, 0:1],
                                    stats_sb[:, 0:1], op=ALU.mult)
            var_sb = sb.tile([P, 1], F32)
            nc.vector.scalar_tensor_tensor(var_sb[:], in0=s1sq[:],
                                           scalar=-1.0 / N,
                                           in1=stats_sb[:, 1:2],
                                           op0=ALU.mult, op1=ALU.add)
            nc.vector.tensor_scalar_mul(var_sb[:], var_sb[:], 1.0 / (N - 1))
            # std = sqrt(var) + 1e-6 via exp(0.5 ln var); rneg = -1/std
            lnv = sb.tile([P, 1], F32)
            nc.scalar.activation(lnv[:], var_sb[:], AF.Ln)
            stdv = sb.tile([P, 1], F32)
            nc.scalar.activation(stdv[:], lnv[:], AF.Exp, scale=0.5)
            nc.vector.tensor_scalar_add(stdv[:], stdv[:], 1e-6)
            rneg = sb.tile([P, 1], F32)
            nc.vector.reciprocal(rneg[:], stdv[:])
            nc.vector.tensor_scalar_mul(rneg[:], rneg[:], -1.0)

            # ---- gate in column layout; q' = q * (1 + sigmoid(dL/std)) ----
            dlqc = sb.tile([P, NQT], F32)
            nc.vector.tensor_scalar(dlqc[:], psum_q[:, :, 3], mu_sb[:, 0:1],
                                    None, op0=ALU.subtract)
            nc.vector.scalar_tensor_tensor(dlqc[:], in0=dlqc[:], scalar=-1.0,
                                           in1=dlqc[:], op0=ALU.mult,
                                           op1=ALU.max)
            eg = sb.tile([P, NQT], F32)
            nc.scalar.activation(eg[:], dlqc[:], AF.Exp, scale=rneg[:])
            nc.vector.tensor_scalar_add(eg[:], eg[:], 1.0)
            opg = sb.tile([P, NQT], F32)
            nc.vector.reciprocal(opg[:], eg[:])
            nc.vector.tensor_scalar_add(opg[:], opg[:], 1.0)  # 1 + gate

            qp_cols = sb.tile([P, 3, NQT], F32)
            for h in range(3):
                nc.vector.tensor_tensor(qp_cols[:, h, :], psum_q[:, :, h],
                                        opg[:], op=ALU.mult)
            if debug:
                nc.sync.dma_start(out=dbg_qp[:],
                                  in_=qp_cols[:].rearrange("p h q -> p (h q)"))

            # q' columns -> row layout in DRAM (transposing DMA), then SBUF
            qrow_d = dpool.tile([3, NSL], F32)
            qrow_view = bass.AP(
                tensor=qrow_d.tensor, offset=qrow_d.offset,
                ap=[[1, P], [NSL, 3], [P, NQT]],
            )
            nc.sync.dma_start(out=qrow_view, in_=qp_cols[:])

            # ---- per-head: grid build + RBF evaluation ----
            att_sb = singles.tile([P, 3, NQT], F32)
            for h in range(3):
                # grid: gnum/gden via exp(t_j k_m) with m on partitions
                psum_g2 = psg.tile([P, 2], F32, tag="g")
                for mt in range(NMT):
                    s_t = stile.tile([P, T], BF16)
                    nc.scalar.activation(s_t[:], tbc_sb[:], AF.Exp,
                                         scale=k_sb[:, mt, h:h + 1])
                    nc.tensor.matmul(psum_g2[:], lhsT=s_t[:],
                                     rhs=lw2_sb[:, mt, h, :],
                                     start=(mt == 0), stop=(mt == NMT - 1))
                g2_sb = sb.tile([P, 2], BF16)
                nc.vector.tensor_copy(g2_sb[:], psum_g2[:])
                if debug:
                    g2f = sb.tile([P, 2], F32)
                    nc.vector.tensor_copy(g2f[:], psum_g2[:])
                    nc.sync.dma_start(out=dbg_g[:, 2 * h:2 * h + 2],
                                      in_=g2f[:])

                # broadcast q' row across partitions via TensorE, then
                # W[j, n] = exp(-((q_n - t_j) / (sig sqrt2))^2)
                qrow_h = sb.tile([1, NSL], F32, tag="qrow")
                nc.sync.dma_start(out=qrow_h[:], in_=qrow_d[h:h + 1, :])
                z2 = wtile.tile([P, NSL], F32, tag="z2")
                for half in range(2):
                    psum_qb = psq.tile([P, 1024], F32, tag="qb")
                    for ch in range(2):
                        off = half * 1024 + ch * 512
                        nc.tensor.matmul(
                            psum_qb[:, ch * 512:(ch + 1) * 512],
                            lhsT=ones_1x128[:],
                            rhs=qrow_h[0:1, off:off + 512],
                            start=True, stop=True,
                        )
                    nc.scalar.activation(z2[:, half * 1024:(half + 1) * 1024],
                                         psum_qb[:], AF.Square,
                                         scale=isq_sb[:, 0:1],
                                         bias=tsig_sb[:, 0:1])
                w_sb = wtile.tile([P, NSL], BF16, tag="w")
                nc.scalar.activation(w_sb[:], z2[:], AF.Exp, scale=-1.0)

                # att columns: out[n_chunk, (num, den)] accumulated in one bank
                psum_att = psa.tile([P, 2 * NQT], F32, tag="att")
                for qt in range(NQT):
                    nc.tensor.matmul(psum_att[:, 2 * qt:2 * qt + 2],
                                     lhsT=w_sb[:, qt * P:(qt + 1) * P],
                                     rhs=g2_sb[:],
                                     start=True, stop=True)
                if debug:
                    ndf = sb.tile([P, 2 * NQT], F32, tag="ndf")
                    nc.vector.tensor_copy(ndf[:], psum_att[:])
                    nc.sync.dma_start(out=dbg_nd[:, h * 2 * NQT:(h + 1) * 2 * NQT],
                                      in_=ndf[:])
                    if h == 2:
                        wf = wtile.tile([P, NSL], F32, tag="wf")
                        nc.vector.tensor_copy(wf[:], w_sb[:])
                        nc.sync.dma_start(out=dbg_w[:], in_=wf[:])
                rden = sb.tile([P, NQT], F32)
                nc.vector.reciprocal(
                    rden[:],
                    psum_att[:].rearrange("p (q two) -> p q two", two=2)[:, :, 1])
                nc.vector.tensor_tensor(
                    att_sb[:, h, :],
                    psum_att[:].rearrange("p (q two) -> p q two", two=2)[:, :, 0],
                    rden[:], op=ALU.mult)

            if debug:
                nc.sync.dma_start(out=dbg_att[:],
                                  in_=att_sb[:].rearrange("p h q -> p (h q)"))

            # ---- output mix (wo), residual, clip -- all in column layout ----
            out_sb = singles.tile([P, 3, NQT], F32)
            for cch in range(3):
                mix = sb.tile([P, NQT], F32, tag="mix")
                nc.vector.tensor_scalar(mix[:], att_sb[:, 0, :],
                                        wocol_sb[:, 3 * cch:3 * cch + 1], None,
                                        op0=ALU.mult)
                for h in (1, 2):
                    nc.vector.scalar_tensor_tensor(
                        mix[:], in0=att_sb[:, h, :],
                        scalar=wocol_sb[:, 3 * cch + h:3 * cch + h + 1],
                        in1=mix[:], op0=ALU.mult, op1=ALU.add)
                nc.vector.tensor_tensor(
                    mix[:], mix[:],
                    qimgT_sb[:, cch * NQT:(cch + 1) * NQT], op=ALU.add)
                nc.vector.tensor_scalar_max(mix[:], mix[:], 0.0)
                nc.vector.tensor_scalar_min(out_sb[:, cch, :], mix[:], 1.0)
            nc.sync.dma_start(out=out[:],
                              in_=out_sb[:].rearrange("p c q -> p (c q)"))

    nc.finalize()
    return nc


_NC_CACHE = {}


def _get_nc(debug=False):
    key = ("dbg" if debug else "nc")
    if key not in _NC_CACHE:
        _NC_CACHE[key] = build_nc(debug)
    return _NC_CACHE[key]


def make_in_maps(rgb, wq, wk, wv, wo):
    x = np.ascontiguousarray(rgb.reshape(4, 3, N)).astype(np.float32)
    lumw = np.array(LUMW, dtype=np.float32)
    wkvl = np.concatenate(
        [wk.T, wv.T, lumw[:, None], np.zeros((3, 1), np.float32)], axis=1
    ).astype(np.float32)
    wql = np.concatenate([wq.T, lumw[:, None]], axis=1).astype(np.float32)
    wocol = np.tile(wo.reshape(1, 9), (P, 1)).astype(np.float32)
    # runtime grid: |q'| <= 2 * max_h sum_c |wq[h,c]| since rgb in [0,1] and
    # (1+gate) <= 2; margin covers the 6-sigma Gaussian quadrature tails
    R = 2.0 * float(np.abs(wq).sum(axis=1).max()) + 1.0
    t0, t1 = -R, R
    hg = (t1 - t0) / (T - 1)
    sig = 1.25 * hg
    isqv = 1.0 / (sig * np.sqrt(2.0))
    tg = (t0 + np.arange(T) * hg).astype(np.float32)
    tbc = np.tile(tg[None, :], (P, 1)).astype(np.float32)
    tsig = (-tg * isqv).reshape(P, 1).astype(np.float32)
    sigk = np.full((P, 1), sig / np.sqrt(2.0), np.float32)
    isq = np.full((P, 1), isqv, np.float32)

    in_maps = []
    for j in range(8):
        b, half = j // 2, j % 2
        sl = slice(half * NSL, (half + 1) * NSL)
        qs = x[b][:, sl]                         # [3, 2048]
        # qimgT[p, c*16+qt] = qs[c, qt*128+p]
        qT = np.ascontiguousarray(
            qs.reshape(3, NQT, P).transpose(2, 0, 1).reshape(P, 3 * NQT))
        in_maps.append({
            "img": x[b],
            "qimg": np.ascontiguousarray(qs),
            "qimgT": qT.astype(np.float32),
            "wkvl": wkvl,
            "wql": wql,
            "wocol": wocol,
            "tbc": tbc,
            "tsig": tsig,
            "sigk": sigk,
            "isq": isq,
        })
    return in_maps


def run(rgb, wq, wk, wv, wo, trace=False, debug=False):
    nc = _get_nc(debug)
    in_maps = make_in_maps(rgb, wq, wk, wv, wo)
    res = run_bass_kernel_spmd(nc, in_maps, core_ids=list(range(8)),
                               trace=trace)
    y = np.zeros((4, 3, N), dtype=np.float32)
    for j in range(8):
        b, half = j // 2, j % 2
        sl = slice(half * NSL, (half + 1) * NSL)
        o = res.results[j]["out"]                # [128, 3*16]
        y[b][:, sl] = o.reshape(P, 3, NQT).transpose(1, 2, 0).reshape(3, NSL)
    return y.reshape(4, 3, 64, 64), res


def kernel(**inputs):
    y, _ = run(inputs["rgb"], inputs["wq"], inputs["wk"], inputs["wv"],
               inputs["wo"])
    return y
